# revision 4
# baseline (speedup 1.0000x reference)
"""NT-Xent (SimCLR) contrastive loss on 8 Trainium2 NeuronCores.

Strategy (matches the row-sharding hint):
  - X = concat(proj_1, proj_2) has 2B=8192 rows, D=256. Core c is assigned
    rows {512c..512c+511} of proj_1 AND of proj_2, so every positive pair
    (i, i+B) lives entirely inside one core's 1024-row shard. The loss is
    invariant under the joint row/column permutation this sharding induces,
    so the gathered Z never needs reordering.
  - Each core L2-normalizes its shard (fp32 stats), casts z to bf16,
    transposes it on the PE (z.T layout is what the matmul needs on both
    operands), and AllGathers the bf16 z.T (512 KB per rank).
  - Main loop: for each of 8 row-tiles x 16 column-chunks, two K=128 bf16
    matmuls accumulate sim into PSUM; a single ScalarE activation computes
    exp(2*sim) in place with a fused free-axis accumulation (the row-sum).
  - Diagonal of exp(sim/T) is exp(2*||z_r||^2) ~= e^2; subtracted as a
    constant inside the final Ln's bias. Positives are computed exactly in
    fp32 from the raw shard (elementwise product of paired halves).
  - Each core emits one partial scalar; the host sums 8 scalars / (2B).
"""

import numpy as np
from contextlib import ExitStack

import concourse.bass as bass
import concourse.tile as tile
from concourse import bacc, mybir
from concourse.bass_utils import run_bass_kernel_spmd
from concourse.masks import make_identity

N_CORES = 8
B = 4096
D = 256            # feature dim; 2 K-chunks of 128
SHARD = 1024       # rows per core (512 from proj_1 + 512 from proj_2)
HALF = SHARD // 2
NT = SHARD // 128  # 8 row-tiles per core
TWO_B = 2 * B      # 8192
NCHUNK = TWO_B // 512  # 16 column chunks of 512
ESCALE = 2.0       # 1 / TEMPERATURE
E2 = float(np.exp(2.0))  # diagonal term exp(sim_rr / T), sim_rr == 1

F32 = mybir.dt.float32
BF16 = mybir.dt.bfloat16

_CACHE = {}


def _build_program():
    nc = bacc.Bacc("TRN2", target_bir_lowering=False, debug=False,
                   num_devices=N_CORES)
    x_in = nc.dram_tensor("x_shard", [SHARD, D], F32, kind="ExternalInput").ap()
    loss_out = nc.dram_tensor("loss_part", [1, 1], F32, kind="ExternalOutput").ap()

    with tile.TileContext(nc) as tc, ExitStack() as ctx:
        sb = ctx.enter_context(tc.tile_pool(name="sb", bufs=1))
        xpool = ctx.enter_context(tc.tile_pool(name="xpool", bufs=NT))
        zpool = ctx.enter_context(tc.tile_pool(name="zpool", bufs=NT))
        tmp = ctx.enter_context(tc.tile_pool(name="tmp", bufs=2))
        tp_ps = ctx.enter_context(
            tc.tile_pool(name="tp_ps", bufs=2, space="PSUM"))
        mm_ps = ctx.enter_context(
            tc.tile_pool(name="mm_ps", bufs=4, space="PSUM"))
        fin_ps = ctx.enter_context(
            tc.tile_pool(name="fin_ps", bufs=1, space="PSUM"))
        dram = ctx.enter_context(tc.tile_pool(name="dram", bufs=1, space="DRAM"))

        # ---- stage 1: load shard, row stats, normalize to bf16 ----
        xs = []
        for t in range(NT):
            xt = xpool.tile([128, D], F32, name=f"x{t}")
            nc.sync.dma_start(xt[:], x_in[t * 128:(t + 1) * 128, :])
            xs.append(xt)

        ssq = sb.tile([128, NT], F32)     # row sum of squares, col t = tile t
        for t in range(NT):
            sqd = tmp.tile([128, D], F32, tag="sqd")
            nc.scalar.activation(sqd[:], xs[t][:],
                                 mybir.ActivationFunctionType.Square,
                                 accum_out=ssq[:, t:t + 1])

        # rn = 1/sqrt(ssq) via exp(-0.5 * ln(ssq)); Exp+Ln share a table set.
        lssq = sb.tile([128, NT], F32)
        rn = sb.tile([128, NT], F32)
        nc.scalar.activation(lssq[:], ssq[:], mybir.ActivationFunctionType.Ln)
        nc.scalar.activation(rn[:], lssq[:], mybir.ActivationFunctionType.Exp,
                             scale=-0.5)

        zs = []
        for t in range(NT):
            zt = zpool.tile([128, D], BF16, name=f"z{t}")
            nc.vector.tensor_scalar_mul(zt[:], xs[t][:], rn[:, t:t + 1])
            zs.append(zt)

        # ---- positives: sum_d x[t] * x[t+4], exact in fp32 ----
        rawpos = sb.tile([128, NT // 2], F32)
        for t in range(NT // 2):
            prod = tmp.tile([128, D], F32, tag="prod")
            nc.vector.tensor_mul(prod[:], xs[t][:], xs[t + NT // 2][:])
            nc.vector.reduce_sum(rawpos[:, t:t + 1], prod[:],
                                 axis=mybir.AxisListType.X)
        posb = sb.tile([128, NT // 2], F32)
        nc.vector.tensor_mul(posb[:], rawpos[:], rn[:, 0:NT // 2])
        nc.vector.tensor_mul(posb[:], posb[:], rn[:, NT // 2:NT])
        possum = sb.tile([128, 1], F32)
        nc.vector.reduce_sum(possum[:], posb[:], axis=mybir.AxisListType.X)

        # ---- stage 2: transpose z shard -> zT_own [256, 1024] bf16 ----
        ident = sb.tile([128, 128], BF16)
        make_identity(nc, ident[:])
        zT_own = [sb.tile([128, SHARD], BF16, name=f"zT_own{k}")
                  for k in range(2)]
        for t in range(NT):
            for k in range(2):
                tp = tp_ps.tile([128, 128], BF16, tag="tp")
                nc.tensor.transpose(tp[:], zs[t][:, k * 128:(k + 1) * 128],
                                    ident[:])
                nc.vector.tensor_copy(zT_own[k][:, t * 128:(t + 1) * 128],
                                      tp[:])

        # ---- stage 3: AllGather bf16 zT across the 8 cores ----
        ag_in = dram.tile([2 * 128, SHARD], BF16)
        ag_out = dram.tile([N_CORES * 2 * 128, SHARD], BF16,
                           addr_space="Shared")
        for k in range(2):
            nc.gpsimd.dma_start(ag_in[k * 128:(k + 1) * 128, :], zT_own[k][:])
        nc.gpsimd.collective_compute(
            "AllGather",
            mybir.AluOpType.bypass,
            replica_groups=[list(range(N_CORES))],
            ins=[ag_in.opt()],
            outs=[ag_out.opt()],
        )
        zt_full = [sb.tile([128, TWO_B], BF16, name=f"zt_full{k}")
                   for k in range(2)]
        for r in range(N_CORES):
            base = r * 256
            for k in range(2):
                nc.gpsimd.dma_start(
                    zt_full[k][:, r * SHARD:(r + 1) * SHARD],
                    ag_out[base + k * 128:base + (k + 1) * 128, :])

        # ---- stage 4: sim chunks + fused exp/rowsum ----
        dsum = sb.tile([128, NT * NCHUNK], F32)
        for m in range(NT):
            lhs0 = zT_own[0][:, m * 128:(m + 1) * 128]
            lhs1 = zT_own[1][:, m * 128:(m + 1) * 128]
            for n in range(NCHUNK):
                ps = mm_ps.tile([128, 512], F32, tag="mm")
                nc.tensor.matmul(ps[:], lhs0, zt_full[0][:, n * 512:(n + 1) * 512],
                                 start=True, stop=False)
                nc.tensor.matmul(ps[:], lhs1, zt_full[1][:, n * 512:(n + 1) * 512],
                                 start=False, stop=True)
                nc.scalar.activation(ps[:], ps[:],
                                     mybir.ActivationFunctionType.Exp,
                                     scale=ESCALE,
                                     accum_out=dsum[:, m * NCHUNK + n:
                                                    m * NCHUNK + n + 1])

        # ---- stage 5: per-row loss and partial reduction ----
        srow = sb.tile([128, NT], F32)
        nc.vector.reduce_sum(srow[:],
                             dsum[:].rearrange("p (m n) -> p m n", n=NCHUNK),
                             axis=mybir.AxisListType.X)
        neg_e2 = sb.tile([128, 1], F32)
        nc.gpsimd.memset(neg_e2[:], -E2)
        lnrow = sb.tile([128, NT], F32)
        nc.scalar.activation(lnrow[:], srow[:],
                             mybir.ActivationFunctionType.Ln, bias=neg_e2[:])
        lnsum = sb.tile([128, 1], F32)
        nc.vector.reduce_sum(lnsum[:], lnrow[:], axis=mybir.AxisListType.X)
        total = sb.tile([128, 1], F32)
        nc.vector.tensor_scalar_mul(total[:], possum[:], -4.0)
        nc.vector.tensor_add(total[:], total[:], lnsum[:])

        ones = sb.tile([128, 1], F32)
        nc.gpsimd.memset(ones[:], 1.0)
        ps1 = fin_ps.tile([1, 1], F32)
        nc.tensor.matmul(ps1[:], ones[:], total[:], start=True, stop=True)
        out_sb = sb.tile([1, 1], F32)
        nc.vector.tensor_copy(out_sb[:], ps1[:])
        nc.sync.dma_start(loss_out[:], out_sb[:])

    nc.compile()
    return nc


def _get_program():
    if "nc" not in _CACHE:
        _CACHE["nc"] = _build_program()
    return _CACHE["nc"]


def kernel(**inputs):
    proj_1 = np.asarray(inputs["proj_1"], dtype=np.float32)
    proj_2 = np.asarray(inputs["proj_2"], dtype=np.float32)
    nc = _get_program()
    in_maps = []
    for c in range(N_CORES):
        shard = np.concatenate(
            [proj_1[c * HALF:(c + 1) * HALF], proj_2[c * HALF:(c + 1) * HALF]],
            axis=0).astype(np.float32)
        in_maps.append({"x_shard": np.ascontiguousarray(shard)})
    res = run_bass_kernel_spmd(nc, in_maps, list(range(N_CORES)))
    total = 0.0
    for c in range(N_CORES):
        total += float(res.results[c]["loss_part"][0, 0])
    return np.float32(total / TWO_B)


# revision 5
# speedup vs baseline: 1.7100x; 1.7100x over previous
"""NT-Xent (SimCLR) contrastive loss on 8 Trainium2 NeuronCores.

Two-launch row-sharded design (no on-device collective: a profiled
AllGather pays a ~50us cross-core start-skew barrier + ~27us transfer,
so the gather runs on the host between two short NEFF launches):

  Launch A (per core, 1/8 of rows): core c gets rows {512c..512c+511}
  of proj_1 AND proj_2, so every positive pair (i, i+B) is core-local
  and the loss is invariant under the induced row/col permutation.
  Normalize in fp32 (rn = exp(-0.5 ln(sum x^2))), cast z to bf16,
  PE-transpose to z.T [256, 1024], emit it plus the exact fp32 sum of
  positive-pair dot products.

  Host: concatenate the 8 z.T chunks -> [256, 8192] bf16.

  Launch B (per core): own z.T block as stationary, full z.T as moving;
  8 row-tiles x 4 super-chunks of [128, 2048] PSUM (4 banks); two K=128
  bf16 matmuls per 512-slice; ONE ScalarE activation per super-chunk
  computes exp(2*sim) in place with fused accumulation (row-sum over
  2048). Diagonal exp(sim_rr/T) ~= e^2 subtracted inside Ln's bias.
  PE ones-matmul folds 128 partitions -> one partial scalar per core.

  Host: loss = (sum ln-parts - 4 * sum positive-parts) / 2B.
"""

import numpy as np
from contextlib import ExitStack

import concourse.bass as bass
import concourse.tile as tile
from concourse import bacc, mybir
from concourse.bass_utils import run_bass_kernel_spmd
from concourse.masks import make_identity

N_CORES = 8
B = 4096
D = 256              # feature dim; 2 K-chunks of 128
SHARD = 1024         # rows per core (512 from proj_1 + 512 from proj_2)
HALF = SHARD // 2
NT = SHARD // 128    # 8 row-tiles per core
TWO_B = 2 * B        # 8192
SUPER = 2048         # ACT super-chunk width (4 PSUM banks)
NSUPER = TWO_B // SUPER  # 4
ESCALE = 2.0         # 1 / TEMPERATURE
E2 = float(np.exp(2.0))  # diagonal term exp(sim_rr / T), sim_rr == 1

F32 = mybir.dt.float32
BF16 = mybir.dt.bfloat16

_CACHE = {}


def _new_nc():
    return bacc.Bacc("TRN2", target_bir_lowering=False, debug=False,
                     num_devices=N_CORES)


def _build_prep():
    """Launch A: x_shard [1024,256] f32 -> zt_chunk [256,1024] bf16,
    pos_part [1,1] f32 (sum over pairs of z_i . z_{i+B}, fp32-exact)."""
    nc = _new_nc()
    x_in = nc.dram_tensor("x_shard", [SHARD, D], F32, kind="ExternalInput").ap()
    zt_out = nc.dram_tensor("zt_chunk", [2 * 128, SHARD], BF16,
                            kind="ExternalOutput").ap()
    pos_out = nc.dram_tensor("pos_part", [1, 1], F32, kind="ExternalOutput").ap()

    with tile.TileContext(nc) as tc, ExitStack() as ctx:
        sb = ctx.enter_context(tc.tile_pool(name="sb", bufs=1))
        xpool = ctx.enter_context(tc.tile_pool(name="xpool", bufs=NT))
        zpool = ctx.enter_context(tc.tile_pool(name="zpool", bufs=NT))
        tmp = ctx.enter_context(tc.tile_pool(name="tmp", bufs=2))
        ps = ctx.enter_context(tc.tile_pool(name="ps", bufs=2, space="PSUM"))

        xs = []
        for t in range(NT):
            xt = xpool.tile([128, D], F32, name=f"x{t}")
            nc.sync.dma_start(xt[:], x_in[t * 128:(t + 1) * 128, :])
            xs.append(xt)

        # row sums of squares on DVE (keeps ACT to the Ln/Exp table set)
        ssq = sb.tile([128, NT], F32)
        for t in range(NT):
            sqd = tmp.tile([128, D], F32, tag="sqd")
            nc.vector.tensor_mul(sqd[:], xs[t][:], xs[t][:])
            nc.vector.reduce_sum(ssq[:, t:t + 1], sqd[:],
                                 axis=mybir.AxisListType.X)
        lssq = sb.tile([128, NT], F32)
        rn = sb.tile([128, NT], F32)
        nc.scalar.activation(lssq[:], ssq[:], mybir.ActivationFunctionType.Ln)
        nc.scalar.activation(rn[:], lssq[:], mybir.ActivationFunctionType.Exp,
                             scale=-0.5)

        zs = []
        for t in range(NT):
            zt = zpool.tile([128, D], BF16, name=f"z{t}")
            nc.vector.tensor_scalar_mul(zt[:], xs[t][:], rn[:, t:t + 1])
            zs.append(zt)

        # positives: fp32-exact sum over pairs
        rawpos = sb.tile([128, NT // 2], F32)
        for t in range(NT // 2):
            prod = tmp.tile([128, D], F32, tag="prod")
            nc.vector.tensor_mul(prod[:], xs[t][:], xs[t + NT // 2][:])
            nc.vector.reduce_sum(rawpos[:, t:t + 1], prod[:],
                                 axis=mybir.AxisListType.X)
        posb = sb.tile([128, NT // 2], F32)
        nc.vector.tensor_mul(posb[:], rawpos[:], rn[:, 0:NT // 2])
        nc.vector.tensor_mul(posb[:], posb[:], rn[:, NT // 2:NT])
        possum = sb.tile([128, 1], F32)
        nc.vector.reduce_sum(possum[:], posb[:], axis=mybir.AxisListType.X)
        ones = sb.tile([128, 1], F32)
        nc.gpsimd.memset(ones[:], 1.0)
        psp = ps.tile([1, 1], F32, tag="fin")
        nc.tensor.matmul(psp[:], ones[:], possum[:], start=True, stop=True)
        pos_sb = sb.tile([1, 1], F32)
        nc.vector.tensor_copy(pos_sb[:], psp[:])
        nc.sync.dma_start(pos_out[:], pos_sb[:])

        # transpose z -> z.T and store
        ident = sb.tile([128, 128], BF16)
        make_identity(nc, ident[:])
        zT = [sb.tile([128, SHARD], BF16, name=f"zT{k}") for k in range(2)]
        for t in range(NT):
            for k in range(2):
                tp = ps.tile([128, 128], BF16, tag="tp")
                nc.tensor.transpose(tp[:], zs[t][:, k * 128:(k + 1) * 128],
                                    ident[:])
                nc.vector.tensor_copy(zT[k][:, t * 128:(t + 1) * 128], tp[:])
        for k in range(2):
            nc.sync.dma_start(zt_out[k * 128:(k + 1) * 128, :], zT[k][:])

    nc.compile()
    return nc


def _build_main():
    """Launch B: zt_own [256,1024] + zt_full [256,8192] bf16 ->
    loss_part [1,1] f32 = sum over own rows of ln(rowsum exp(2 sim) - e^2)."""
    nc = _new_nc()
    own_in = nc.dram_tensor("zt_own", [2 * 128, SHARD], BF16,
                            kind="ExternalInput").ap()
    full_in = nc.dram_tensor("zt_full", [2 * 128, TWO_B], BF16,
                             kind="ExternalInput").ap()
    loss_out = nc.dram_tensor("loss_part", [1, 1], F32,
                              kind="ExternalOutput").ap()

    with tile.TileContext(nc) as tc, ExitStack() as ctx:
        sb = ctx.enter_context(tc.tile_pool(name="sb", bufs=1))
        mm_ps = ctx.enter_context(tc.tile_pool(name="mm_ps", bufs=2,
                                               space="PSUM"))

        zown = [sb.tile([128, SHARD], BF16, name=f"zown{k}") for k in range(2)]
        for k in range(2):
            nc.sync.dma_start(zown[k][:], own_in[k * 128:(k + 1) * 128, :])
        # full z.T split into per-super-chunk tiles so matmuls can start
        # as soon as their column block arrives (alternating DMA queues)
        zfull = {}
        for k in range(2):
            for j in range(NSUPER):
                zt = sb.tile([128, SUPER], BF16, name=f"zfull{k}_{j}")
                eng = nc.sync if (k * NSUPER + j) % 2 == 0 else nc.gpsimd
                eng.dma_start(zt[:], full_in[k * 128:(k + 1) * 128,
                                             j * SUPER:(j + 1) * SUPER])
                zfull[(k, j)] = zt

        dsum = sb.tile([128, NT * NSUPER], F32)
        for m in range(NT):
            lhs = [zown[k][:, m * 128:(m + 1) * 128] for k in range(2)]
            for j in range(NSUPER):
                ps = mm_ps.tile([128, SUPER], F32, tag="mm")
                for k in range(2):
                    for s in range(4):
                        nc.tensor.matmul(ps[:, s * 512:(s + 1) * 512],
                                         lhs[k],
                                         zfull[(k, j)][:, s * 512:(s + 1) * 512],
                                         start=(k == 0), stop=(k == 1))
                idx = m * NSUPER + j
                nc.scalar.activation(ps[:], ps[:],
                                     mybir.ActivationFunctionType.Exp,
                                     scale=ESCALE,
                                     accum_out=dsum[:, idx:idx + 1])

        srow = sb.tile([128, NT], F32)
        nc.vector.reduce_sum(srow[:],
                             dsum[:].rearrange("p (m j) -> p m j", j=NSUPER),
                             axis=mybir.AxisListType.X)
        neg_e2 = sb.tile([128, 1], F32)
        nc.gpsimd.memset(neg_e2[:], -E2)
        lnrow = sb.tile([128, NT], F32)
        nc.scalar.activation(lnrow[:], srow[:],
                             mybir.ActivationFunctionType.Ln, bias=neg_e2[:])
        lnsum = sb.tile([128, 1], F32)
        nc.vector.reduce_sum(lnsum[:], lnrow[:], axis=mybir.AxisListType.X)

        ones = sb.tile([128, 1], F32)
        nc.gpsimd.memset(ones[:], 1.0)
        ps1 = mm_ps.tile([1, 1], F32, tag="mm")
        nc.tensor.matmul(ps1[:], ones[:], lnsum[:], start=True, stop=True)
        out_sb = sb.tile([1, 1], F32)
        nc.vector.tensor_copy(out_sb[:], ps1[:])
        nc.sync.dma_start(loss_out[:], out_sb[:])

    nc.compile()
    return nc


def _get_programs():
    if "prep" not in _CACHE:
        _CACHE["prep"] = _build_prep()
        _CACHE["main"] = _build_main()
    return _CACHE["prep"], _CACHE["main"]


def shard_inputs(proj_1, proj_2):
    in_maps = []
    for c in range(N_CORES):
        shard = np.concatenate(
            [proj_1[c * HALF:(c + 1) * HALF], proj_2[c * HALF:(c + 1) * HALF]],
            axis=0).astype(np.float32)
        in_maps.append({"x_shard": np.ascontiguousarray(shard)})
    return in_maps


def main_inputs(prep_results):
    zt_full = np.concatenate(
        [prep_results[c]["zt_chunk"] for c in range(N_CORES)], axis=1)
    zt_full = np.ascontiguousarray(zt_full)
    return [{"zt_own": np.ascontiguousarray(prep_results[c]["zt_chunk"]),
             "zt_full": zt_full} for c in range(N_CORES)]


def kernel(**inputs):
    proj_1 = np.asarray(inputs["proj_1"], dtype=np.float32)
    proj_2 = np.asarray(inputs["proj_2"], dtype=np.float32)
    nc_prep, nc_main = _get_programs()
    core_ids = list(range(N_CORES))

    res_a = run_bass_kernel_spmd(nc_prep, shard_inputs(proj_1, proj_2),
                                 core_ids)
    res_b = run_bass_kernel_spmd(nc_main, main_inputs(res_a.results), core_ids)

    total = 0.0
    for c in range(N_CORES):
        total += float(res_b.results[c]["loss_part"][0, 0])
        total += -4.0 * float(res_a.results[c]["pos_part"][0, 0])
    return np.float32(total / TWO_B)


# revision 6
# speedup vs baseline: 1.7902x; 1.0469x over previous
"""NT-Xent (SimCLR) contrastive loss on 8 Trainium2 NeuronCores.

Two-launch row-sharded design (no on-device collective: a profiled
AllGather pays a ~50us cross-core start-skew barrier + ~27us transfer,
so the gather runs on the host between two short NEFF launches):

  Launch A (per core, 1/8 of rows): core c gets rows {512c..512c+511}
  of proj_1 AND proj_2, so every positive pair (i, i+B) is core-local
  and the loss is invariant under the induced row/col permutation.
  Normalize in fp32 (rn = exp(-0.5 ln(sum x^2))), cast z to bf16,
  PE-transpose to z.T [256, 1024], emit it plus the exact fp32 sum of
  positive-pair dot products.

  Host: concatenate the 8 z.T chunks -> [256, 8192] bf16.

  Launch B (per core): own z.T block as stationary, full z.T as moving;
  8 row-tiles x 4 super-chunks of [128, 2048] PSUM (4 banks); two K=128
  bf16 matmuls per 512-slice; ONE ScalarE activation per super-chunk
  computes exp(2*sim) in place with fused accumulation (row-sum over
  2048). Diagonal exp(sim_rr/T) ~= e^2 subtracted inside Ln's bias.
  PE ones-matmul folds 128 partitions -> one partial scalar per core.

  Host: loss = (sum ln-parts - 4 * sum positive-parts) / 2B.
"""

import numpy as np
from contextlib import ExitStack

import concourse.bass as bass
import concourse.tile as tile
from concourse import bacc, mybir
from concourse.bass_utils import run_bass_kernel_spmd
from concourse.masks import make_identity

N_CORES = 8
B = 4096
D = 256              # feature dim; 2 K-chunks of 128
SHARD = 1024         # rows per core (512 from proj_1 + 512 from proj_2)
HALF = SHARD // 2
NT = SHARD // 128    # 8 row-tiles per core
TWO_B = 2 * B        # 8192
SUPER = 2048         # ACT super-chunk width (4 PSUM banks)
NSUPER = TWO_B // SUPER  # 4
ESCALE = 2.0         # 1 / TEMPERATURE
E2 = float(np.exp(2.0))  # diagonal term exp(sim_rr / T), sim_rr == 1

F32 = mybir.dt.float32
BF16 = mybir.dt.bfloat16

_CACHE = {}


def _new_nc():
    return bacc.Bacc("TRN2", target_bir_lowering=False, debug=False,
                     num_devices=N_CORES)


def _build_prep():
    """Launch A: x_shard [1024,256] f32 -> zt_chunk [256,1024] bf16,
    pos_part [1,1] f32 (sum over pairs of z_i . z_{i+B}, fp32-exact)."""
    nc = _new_nc()
    x_in = nc.dram_tensor("x_shard", [SHARD, D], F32, kind="ExternalInput").ap()
    zt_out = nc.dram_tensor("zt_chunk", [2 * 128, SHARD], BF16,
                            kind="ExternalOutput").ap()
    pos_out = nc.dram_tensor("pos_part", [1, 1], F32, kind="ExternalOutput").ap()

    with tile.TileContext(nc) as tc, ExitStack() as ctx:
        sb = ctx.enter_context(tc.tile_pool(name="sb", bufs=1))
        xpool = ctx.enter_context(tc.tile_pool(name="xpool", bufs=NT))
        zpool = ctx.enter_context(tc.tile_pool(name="zpool", bufs=NT))
        tmp = ctx.enter_context(tc.tile_pool(name="tmp", bufs=2))
        ps = ctx.enter_context(tc.tile_pool(name="ps", bufs=2, space="PSUM"))

        xs = []
        for t in range(NT):
            xt = xpool.tile([128, D], F32, name=f"x{t}")
            nc.sync.dma_start(xt[:], x_in[t * 128:(t + 1) * 128, :])
            xs.append(xt)

        # row sums of squares on DVE (keeps ACT to the Ln/Exp table set)
        ssq = sb.tile([128, NT], F32)
        for t in range(NT):
            sqd = tmp.tile([128, D], F32, tag="sqd")
            nc.vector.affine_mul_reduce(out=sqd[:], accum_out=ssq[:, t:t + 1],
                                        in0=xs[t][:], in1=xs[t][:],
                                        scale=1.0, bias=0.0)
        lssq = sb.tile([128, NT], F32)
        rn = sb.tile([128, NT], F32)
        nc.scalar.activation(lssq[:], ssq[:], mybir.ActivationFunctionType.Ln)
        nc.scalar.activation(rn[:], lssq[:], mybir.ActivationFunctionType.Exp,
                             scale=-0.5)

        zs = []
        for t in range(NT):
            zt = zpool.tile([128, D], BF16, name=f"z{t}")
            nc.scalar.mul(zt[:], xs[t][:], rn[:, t:t + 1])
            zs.append(zt)

        # positives: fp32-exact sum over pairs
        rawpos = sb.tile([128, NT // 2], F32)
        for t in range(NT // 2):
            prod = tmp.tile([128, D], F32, tag="prod")
            nc.vector.affine_mul_reduce(out=prod[:],
                                        accum_out=rawpos[:, t:t + 1],
                                        in0=xs[t][:], in1=xs[t + NT // 2][:],
                                        scale=1.0, bias=0.0)
        posb = sb.tile([128, NT // 2], F32)
        nc.vector.tensor_mul(posb[:], rawpos[:], rn[:, 0:NT // 2])
        nc.vector.tensor_mul(posb[:], posb[:], rn[:, NT // 2:NT])
        possum = sb.tile([128, 1], F32)
        nc.vector.reduce_sum(possum[:], posb[:], axis=mybir.AxisListType.X)
        ones = sb.tile([128, 1], F32)
        nc.gpsimd.memset(ones[:], 1.0)
        psp = ps.tile([1, 1], F32, tag="fin")
        nc.tensor.matmul(psp[:], ones[:], possum[:], start=True, stop=True)
        pos_sb = sb.tile([1, 1], F32)
        nc.vector.tensor_copy(pos_sb[:], psp[:])
        nc.sync.dma_start(pos_out[:], pos_sb[:])

        # transpose z -> z.T and store
        ident = sb.tile([128, 128], BF16)
        make_identity(nc, ident[:])
        zT = [sb.tile([128, SHARD], BF16, name=f"zT{k}") for k in range(2)]
        for t in range(NT):
            for k in range(2):
                tp = ps.tile([128, 128], BF16, tag="tp")
                nc.tensor.transpose(tp[:], zs[t][:, k * 128:(k + 1) * 128],
                                    ident[:])
                dst = zT[k][:, t * 128:(t + 1) * 128]
                if (t + k) % 2 == 0:
                    nc.vector.tensor_copy(dst, tp[:])
                else:
                    nc.scalar.copy(dst, tp[:])
        for k in range(2):
            nc.sync.dma_start(zt_out[k * 128:(k + 1) * 128, :], zT[k][:])

    nc.compile()
    return nc


def _build_main():
    """Launch B: zt_own [256,1024] + zt_full [256,8192] bf16 ->
    loss_part [1,1] f32 = sum over own rows of ln(rowsum exp(2 sim) - e^2)."""
    nc = _new_nc()
    own_in = nc.dram_tensor("zt_own", [2 * 128, SHARD], BF16,
                            kind="ExternalInput").ap()
    full_in = nc.dram_tensor("zt_full", [2 * 128, TWO_B], BF16,
                             kind="ExternalInput").ap()
    loss_out = nc.dram_tensor("loss_part", [1, 1], F32,
                              kind="ExternalOutput").ap()

    with tile.TileContext(nc) as tc, ExitStack() as ctx:
        sb = ctx.enter_context(tc.tile_pool(name="sb", bufs=1))
        mm_ps = ctx.enter_context(tc.tile_pool(name="mm_ps", bufs=2,
                                               space="PSUM"))

        zown = [sb.tile([128, SHARD], BF16, name=f"zown{k}") for k in range(2)]
        for k in range(2):
            nc.sync.dma_start(zown[k][:], own_in[k * 128:(k + 1) * 128, :])
        # full z.T in [128,1024] tiles, ordered by column so the first
        # matmuls start as soon as their block lands (HWDGE queue)
        NQ = TWO_B // 1024
        zq = {}
        for j2 in range(NQ):
            for k in range(2):
                zt = sb.tile([128, 1024], BF16, name=f"zq{k}_{j2}")
                nc.sync.dma_start(zt[:], full_in[k * 128:(k + 1) * 128,
                                                 j2 * 1024:(j2 + 1) * 1024])
                zq[(k, j2)] = zt

        dsum = sb.tile([128, NT * NSUPER], F32)
        for m in range(NT):
            lhs = [zown[k][:, m * 128:(m + 1) * 128] for k in range(2)]
            for j in range(NSUPER):
                ps = mm_ps.tile([128, SUPER], F32, tag="mm")
                for k in range(2):
                    for s in range(4):
                        src_t = zq[(k, 2 * j + s // 2)]
                        nc.tensor.matmul(ps[:, s * 512:(s + 1) * 512],
                                         lhs[k],
                                         src_t[:, (s % 2) * 512:(s % 2 + 1) * 512],
                                         start=(k == 0), stop=(k == 1))
                idx = m * NSUPER + j
                nc.scalar.activation(ps[:], ps[:],
                                     mybir.ActivationFunctionType.Exp,
                                     scale=ESCALE,
                                     accum_out=dsum[:, idx:idx + 1])

        srow = sb.tile([128, NT], F32)
        nc.vector.reduce_sum(srow[:],
                             dsum[:].rearrange("p (m j) -> p m j", j=NSUPER),
                             axis=mybir.AxisListType.X)
        neg_e2 = sb.tile([128, 1], F32)
        nc.gpsimd.memset(neg_e2[:], -E2)
        lnrow = sb.tile([128, NT], F32)
        nc.scalar.activation(lnrow[:], srow[:],
                             mybir.ActivationFunctionType.Ln, bias=neg_e2[:])
        lnsum = sb.tile([128, 1], F32)
        nc.vector.reduce_sum(lnsum[:], lnrow[:], axis=mybir.AxisListType.X)

        ones = sb.tile([128, 1], F32)
        nc.gpsimd.memset(ones[:], 1.0)
        ps1 = mm_ps.tile([1, 1], F32, tag="mm")
        nc.tensor.matmul(ps1[:], ones[:], lnsum[:], start=True, stop=True)
        out_sb = sb.tile([1, 1], F32)
        nc.vector.tensor_copy(out_sb[:], ps1[:])
        nc.sync.dma_start(loss_out[:], out_sb[:])

    nc.compile()
    return nc


def _get_programs():
    if "prep" not in _CACHE:
        _CACHE["prep"] = _build_prep()
        _CACHE["main"] = _build_main()
    return _CACHE["prep"], _CACHE["main"]


def shard_inputs(proj_1, proj_2):
    in_maps = []
    for c in range(N_CORES):
        shard = np.concatenate(
            [proj_1[c * HALF:(c + 1) * HALF], proj_2[c * HALF:(c + 1) * HALF]],
            axis=0).astype(np.float32)
        in_maps.append({"x_shard": np.ascontiguousarray(shard)})
    return in_maps


def main_inputs(prep_results):
    zt_full = np.concatenate(
        [prep_results[c]["zt_chunk"] for c in range(N_CORES)], axis=1)
    zt_full = np.ascontiguousarray(zt_full)
    return [{"zt_own": np.ascontiguousarray(prep_results[c]["zt_chunk"]),
             "zt_full": zt_full} for c in range(N_CORES)]


def kernel(**inputs):
    proj_1 = np.asarray(inputs["proj_1"], dtype=np.float32)
    proj_2 = np.asarray(inputs["proj_2"], dtype=np.float32)
    nc_prep, nc_main = _get_programs()
    core_ids = list(range(N_CORES))

    res_a = run_bass_kernel_spmd(nc_prep, shard_inputs(proj_1, proj_2),
                                 core_ids)
    res_b = run_bass_kernel_spmd(nc_main, main_inputs(res_a.results), core_ids)

    total = 0.0
    for c in range(N_CORES):
        total += float(res_b.results[c]["loss_part"][0, 0])
        total += -4.0 * float(res_a.results[c]["pos_part"][0, 0])
    return np.float32(total / TWO_B)


# revision 7
# speedup vs baseline: 1.7979x; 1.0043x over previous
"""NT-Xent (SimCLR) contrastive loss on 8 Trainium2 NeuronCores.

Two-launch row-sharded design (no on-device collective: a profiled
AllGather pays a ~50us cross-core start-skew barrier + ~27us transfer,
so the gather runs on the host between two short NEFF launches):

  Launch A (per core, 1/8 of rows): core c gets rows {512c..512c+511}
  of proj_1 AND proj_2, so every positive pair (i, i+B) is core-local
  and the loss is invariant under the induced row/col permutation.
  Normalize in fp32 (rn = exp(-0.5 ln(sum x^2))), cast z to bf16,
  PE-transpose to z.T [256, 1024], emit it plus the exact fp32 sum of
  positive-pair dot products.

  Host: concatenate the 8 z.T chunks -> [256, 8192] bf16.

  Launch B (per core): own z.T block as stationary, full z.T as moving;
  8 row-tiles x 4 super-chunks of [128, 2048] PSUM (4 banks); two K=128
  bf16 matmuls per 512-slice; ONE ScalarE activation per super-chunk
  computes exp(2*sim) in place with fused accumulation (row-sum over
  2048). Diagonal exp(sim_rr/T) ~= e^2 subtracted inside Ln's bias.
  PE ones-matmul folds 128 partitions -> one partial scalar per core.

  Host: loss = (sum ln-parts - 4 * sum positive-parts) / 2B.
"""

import numpy as np
from contextlib import ExitStack

import concourse.bass as bass
import concourse.tile as tile
from concourse import bacc, mybir
from concourse.bass_utils import run_bass_kernel_spmd
from concourse.masks import make_identity

N_CORES = 8
B = 4096
D = 256              # feature dim; 2 K-chunks of 128
SHARD = 1024         # rows per core (512 from proj_1 + 512 from proj_2)
HALF = SHARD // 2
NT = SHARD // 128    # 8 row-tiles per core
TWO_B = 2 * B        # 8192
SUPER = 2048         # ACT super-chunk width (4 PSUM banks)
NSUPER = TWO_B // SUPER  # 4
ESCALE = 2.0         # 1 / TEMPERATURE
E2 = float(np.exp(2.0))  # diagonal term exp(sim_rr / T), sim_rr == 1

F32 = mybir.dt.float32
BF16 = mybir.dt.bfloat16

_CACHE = {}


def _new_nc():
    return bacc.Bacc("TRN2", target_bir_lowering=False, debug=False,
                     num_devices=N_CORES)


def _build_prep():
    """Launch A: x_shard [1024,256] f32 -> zt_chunk [256,1024] bf16,
    pos_part [1,1] f32 (sum over pairs of z_i . z_{i+B}, fp32-exact)."""
    nc = _new_nc()
    x_in = nc.dram_tensor("x_shard", [SHARD, D], F32, kind="ExternalInput").ap()
    zt_out = nc.dram_tensor("zt_chunk", [2 * 128, SHARD], BF16,
                            kind="ExternalOutput").ap()
    pos_out = nc.dram_tensor("pos_part", [1, 1], F32, kind="ExternalOutput").ap()

    with tile.TileContext(nc) as tc, ExitStack() as ctx:
        sb = ctx.enter_context(tc.tile_pool(name="sb", bufs=1))
        xpool = ctx.enter_context(tc.tile_pool(name="xpool", bufs=NT))
        zpool = ctx.enter_context(tc.tile_pool(name="zpool", bufs=NT))
        tmp = ctx.enter_context(tc.tile_pool(name="tmp", bufs=2))
        ps = ctx.enter_context(tc.tile_pool(name="ps", bufs=2, space="PSUM"))

        xs = []
        for t in range(NT):
            xt = xpool.tile([128, D], F32, name=f"x{t}")
            eng = nc.gpsimd if t < NT // 2 else nc.sync
            eng.dma_start(xt[:], x_in[t * 128:(t + 1) * 128, :])
            xs.append(xt)

        # row sums of squares on DVE (keeps ACT to the Ln/Exp table set)
        ssq = sb.tile([128, NT], F32)
        for t in range(NT):
            sqd = tmp.tile([128, D], F32, tag="sqd")
            nc.vector.affine_mul_reduce(out=sqd[:], accum_out=ssq[:, t:t + 1],
                                        in0=xs[t][:], in1=xs[t][:],
                                        scale=1.0, bias=0.0)
        lssq = sb.tile([128, NT], F32)
        rn = sb.tile([128, NT], F32)
        nc.scalar.activation(lssq[:], ssq[:], mybir.ActivationFunctionType.Ln)
        nc.scalar.activation(rn[:], lssq[:], mybir.ActivationFunctionType.Exp,
                             scale=-0.5)

        zs = []
        for t in range(NT):
            zt = zpool.tile([128, D], BF16, name=f"z{t}")
            nc.vector.tensor_scalar_mul(zt[:], xs[t][:], rn[:, t:t + 1])
            zs.append(zt)

        # positives: fp32-exact sum over pairs
        rawpos = sb.tile([128, NT // 2], F32)
        for t in range(NT // 2):
            prod = tmp.tile([128, D], F32, tag="prod")
            nc.vector.affine_mul_reduce(out=prod[:],
                                        accum_out=rawpos[:, t:t + 1],
                                        in0=xs[t][:], in1=xs[t + NT // 2][:],
                                        scale=1.0, bias=0.0)
        posb = sb.tile([128, NT // 2], F32)
        nc.vector.tensor_mul(posb[:], rawpos[:], rn[:, 0:NT // 2])
        nc.vector.tensor_mul(posb[:], posb[:], rn[:, NT // 2:NT])
        possum = sb.tile([128, 1], F32)
        nc.vector.reduce_sum(possum[:], posb[:], axis=mybir.AxisListType.X)
        ones = sb.tile([128, 1], F32)
        nc.gpsimd.memset(ones[:], 1.0)
        psp = ps.tile([1, 1], F32, tag="fin")
        nc.tensor.matmul(psp[:], ones[:], possum[:], start=True, stop=True)
        pos_sb = sb.tile([1, 1], F32)
        nc.vector.tensor_copy(pos_sb[:], psp[:])
        nc.sync.dma_start(pos_out[:], pos_sb[:])

        # transpose z -> z.T and store
        ident = sb.tile([128, 128], BF16)
        make_identity(nc, ident[:])
        zT = [sb.tile([128, SHARD], BF16, name=f"zT{k}") for k in range(2)]
        for t in range(NT):
            for k in range(2):
                tp = ps.tile([128, 128], BF16, tag="tp")
                nc.tensor.transpose(tp[:], zs[t][:, k * 128:(k + 1) * 128],
                                    ident[:])
                dst = zT[k][:, t * 128:(t + 1) * 128]
                if (2 * t + k) % 16 < 10:
                    nc.vector.tensor_copy(dst, tp[:])
                else:
                    nc.scalar.copy(dst, tp[:])
        for k in range(2):
            nc.sync.dma_start(zt_out[k * 128:(k + 1) * 128, :], zT[k][:])

    nc.compile()
    return nc


def _build_main():
    """Launch B: zt_own [256,1024] + zt_full [256,8192] bf16 ->
    loss_part [1,1] f32 = sum over own rows of ln(rowsum exp(2 sim) - e^2)."""
    nc = _new_nc()
    own_in = nc.dram_tensor("zt_own", [2 * 128, SHARD], BF16,
                            kind="ExternalInput").ap()
    full_in = nc.dram_tensor("zt_full", [2 * 128, TWO_B], BF16,
                             kind="ExternalInput").ap()
    loss_out = nc.dram_tensor("loss_part", [1, 1], F32,
                              kind="ExternalOutput").ap()

    with tile.TileContext(nc) as tc, ExitStack() as ctx:
        sb = ctx.enter_context(tc.tile_pool(name="sb", bufs=1))
        mm_ps = ctx.enter_context(tc.tile_pool(name="mm_ps", bufs=2,
                                               space="PSUM"))

        zown = [sb.tile([128, SHARD], BF16, name=f"zown{k}") for k in range(2)]
        for k in range(2):
            nc.sync.dma_start(zown[k][:], own_in[k * 128:(k + 1) * 128, :])
        # full z.T in [128,1024] tiles, ordered by column so the first
        # matmuls start as soon as their block lands (HWDGE queue)
        NQ = TWO_B // 1024
        zq = {}
        for j2 in range(NQ):
            for k in range(2):
                zt = sb.tile([128, 1024], BF16, name=f"zq{k}_{j2}")
                nc.sync.dma_start(zt[:], full_in[k * 128:(k + 1) * 128,
                                                 j2 * 1024:(j2 + 1) * 1024])
                zq[(k, j2)] = zt

        dsum = sb.tile([128, NT * NSUPER], F32)
        for m in range(NT):
            lhs = [zown[k][:, m * 128:(m + 1) * 128] for k in range(2)]
            for j in range(NSUPER):
                ps = mm_ps.tile([128, SUPER], F32, tag="mm")
                for k in range(2):
                    for s in range(4):
                        src_t = zq[(k, 2 * j + s // 2)]
                        nc.tensor.matmul(ps[:, s * 512:(s + 1) * 512],
                                         lhs[k],
                                         src_t[:, (s % 2) * 512:(s % 2 + 1) * 512],
                                         start=(k == 0), stop=(k == 1))
                idx = m * NSUPER + j
                nc.scalar.activation(ps[:], ps[:],
                                     mybir.ActivationFunctionType.Exp,
                                     scale=ESCALE,
                                     accum_out=dsum[:, idx:idx + 1])

        srow = sb.tile([128, NT], F32)
        nc.vector.reduce_sum(srow[:],
                             dsum[:].rearrange("p (m j) -> p m j", j=NSUPER),
                             axis=mybir.AxisListType.X)
        neg_e2 = sb.tile([128, 1], F32)
        nc.gpsimd.memset(neg_e2[:], -E2)
        lnrow = sb.tile([128, NT], F32)
        nc.scalar.activation(lnrow[:], srow[:],
                             mybir.ActivationFunctionType.Ln, bias=neg_e2[:])
        lnsum = sb.tile([128, 1], F32)
        nc.vector.reduce_sum(lnsum[:], lnrow[:], axis=mybir.AxisListType.X)

        ones = sb.tile([128, 1], F32)
        nc.gpsimd.memset(ones[:], 1.0)
        ps1 = mm_ps.tile([1, 1], F32, tag="mm")
        nc.tensor.matmul(ps1[:], ones[:], lnsum[:], start=True, stop=True)
        out_sb = sb.tile([1, 1], F32)
        nc.vector.tensor_copy(out_sb[:], ps1[:])
        nc.sync.dma_start(loss_out[:], out_sb[:])

    nc.compile()
    return nc


def _get_programs():
    if "prep" not in _CACHE:
        _CACHE["prep"] = _build_prep()
        _CACHE["main"] = _build_main()
    return _CACHE["prep"], _CACHE["main"]


def shard_inputs(proj_1, proj_2):
    in_maps = []
    for c in range(N_CORES):
        shard = np.concatenate(
            [proj_1[c * HALF:(c + 1) * HALF], proj_2[c * HALF:(c + 1) * HALF]],
            axis=0).astype(np.float32)
        in_maps.append({"x_shard": np.ascontiguousarray(shard)})
    return in_maps


def main_inputs(prep_results):
    zt_full = np.concatenate(
        [prep_results[c]["zt_chunk"] for c in range(N_CORES)], axis=1)
    zt_full = np.ascontiguousarray(zt_full)
    return [{"zt_own": np.ascontiguousarray(prep_results[c]["zt_chunk"]),
             "zt_full": zt_full} for c in range(N_CORES)]


def kernel(**inputs):
    proj_1 = np.asarray(inputs["proj_1"], dtype=np.float32)
    proj_2 = np.asarray(inputs["proj_2"], dtype=np.float32)
    nc_prep, nc_main = _get_programs()
    core_ids = list(range(N_CORES))

    res_a = run_bass_kernel_spmd(nc_prep, shard_inputs(proj_1, proj_2),
                                 core_ids)
    res_b = run_bass_kernel_spmd(nc_main, main_inputs(res_a.results), core_ids)

    total = 0.0
    for c in range(N_CORES):
        total += float(res_b.results[c]["loss_part"][0, 0])
        total += -4.0 * float(res_a.results[c]["pos_part"][0, 0])
    return np.float32(total / TWO_B)
